# revision 51
# baseline (speedup 1.0000x reference)
"""Trainium2 Bass kernel for DecomposedQkvAttention (fp8-qdq attention).

Sharding: 16 heads / 8 cores = 2 heads per core (both batches on every core).
Each core computes qkv projection for its head slice, fp8-qdq attention in a
"transposed" layout (scores as [k_part, q_free]), and its partial contribution
to the output projection; the host sums the 8 partials and adds b_proj.

v4 (on the v0 schedule skeleton):
  - qkv: q,k in 3-pass bf16 hi/lo (fp8-flip sensitivity needs ~fp32 there),
    v in 1-pass bf16 (its flip noise averages out through the attention sum)
  - softmax z via all-ones [128,128] matmul (broadcasts z to every psum
    partition); 1/z via reciprocal_approx_fast (1 DVE op, ~18 bits)
  - attn@V in fp8 DoubleRow perf mode
  - outT / w_proj / output partials in fp16

Self-contained: hardcodes B=2, S=2048, E=1024, H=16, D=64.
"""
import os
import numpy as np
import ml_dtypes

B, S, E, H = 2, 2048, 1024, 16
D = E // H            # 64
NCORES = 8
HPC = H // NCORES     # heads per core = 2
BS = B * S            # 4096
KO = S // 128         # 16 k-chunks
QB = S // 512         # 4 q-blocks of 512
EO = E // 128         # 8 contraction chunks for qkv proj

_last_results = None  # stash for test harness (exec_time_ns etc.)


def _build_bass():
    import concourse.bass as bass
    import concourse.mybir as mybir
    import concourse.tile as tile
    from concourse import bacc
    from concourse.masks import make_identity

    fp32 = mybir.dt.float32
    bf16 = mybir.dt.bfloat16
    fp16 = mybir.dt.float16
    fp8 = mybir.dt.float8e4
    AF = mybir.ActivationFunctionType

    cfg_evac_n = int(os.environ.get("K_EVAC_N", "6"))    # ACT share numerator
    cfg_evac_d = int(os.environ.get("K_EVAC_D", "8"))    # denominator
    cfg_ps_s = int(os.environ.get("K_PS_S", "4"))
    cfg_ps_o = int(os.environ.get("K_PS_O", "2"))
    cfg_work = int(os.environ.get("K_WORK", "5"))
    cfg_gps_n = int(os.environ.get("K_GPS_N", "0"))      # a8 gps share n/d
    cfg_gps_d = int(os.environ.get("K_GPS_D", "3"))
    cfg_mk_gps = int(os.environ.get("K_MK_GPS", "0"))    # mask-mul gps of 8
    cfg_v1 = bool(int(os.environ.get("K_V1", "1")))      # v in 1-pass bf16
    cfg_dp = bool(int(os.environ.get("K_DP", "0")))      # DoublePixel scores

    nc = bacc.Bacc(
        "TRN2", target_bir_lowering=False, debug=False, num_devices=NCORES,
    )

    xTh = nc.dram_tensor("xTh", [128, EO, BS], bf16, kind="ExternalInput")
    xTl = nc.dram_tensor("xTl", [128, EO, BS], bf16, kind="ExternalInput")
    emT = nc.dram_tensor("emT", [128, KO, S], fp16, kind="ExternalInput")
    w_ah = nc.dram_tensor("w_ah", [128, EO, 3 * 128], bf16, kind="ExternalInput")
    w_al = nc.dram_tensor("w_al", [128, EO, 3 * 128], bf16, kind="ExternalInput")
    b_a = nc.dram_tensor("b_a", [128, 3], fp32, kind="ExternalInput")
    w_pT = nc.dram_tensor("w_pT", [128, E], fp16, kind="ExternalInput")
    out_d = nc.dram_tensor("out_p", [BS, E], fp16, kind="ExternalOutput")

    with tile.TileContext(nc) as tc:
        with tc.tile_pool(name="static", bufs=1) as static:
            w_pT_t = static.tile([128, E], fp16)
            nc.sync.dma_start(w_pT_t, w_pT[:])
            ones128 = static.tile([128, 128], fp16)
            nc.vector.memset(ones128, 1.0)

            # persistent activations (split per batch to unlock P1/P4 overlap)
            q8 = [static.tile([128, S], fp8, name=f"q8_{b}") for b in range(B)]
            k8 = [static.tile([128, S], fp8, name=f"k8_{b}") for b in range(B)]
            v8 = [static.tile([128, KO, 128], fp8, name=f"v8_{b}") for b in range(B)]
            EmT = static.tile([128, KO, S], fp16)     # exp(fp8(mask^T))
            outT = static.tile([128, BS], fp16)       # [(h,d), b*s]

            # ---- P1: qkv projection ----
            with (
                tc.tile_pool(name="xin", bufs=3) as xin,
                tc.tile_pool(name="v32", bufs=1) as v32p,
                tc.tile_pool(name="ps1", bufs=2, space="PSUM") as ps1,
            ):
                w_ah_t = v32p.tile([128, EO, 3 * 128], bf16)
                nc.sync.dma_start(w_ah_t, w_ah[:])
                w_al_t = v32p.tile([128, EO, 3 * 128], bf16)
                nc.sync.dma_start(w_al_t, w_al[:])
                b_a_t = v32p.tile([128, 3], fp32)
                nc.sync.dma_start(b_a_t, b_a[:])
                ident32 = v32p.tile([128, 128], fp32)
                make_identity(nc, ident32)
                v32T = [v32p.tile([128, S], fp32, name=f"v32T_{b}") for b in range(B)]
                with tc.tile_pool(name="ps2", bufs=2, space="PSUM") as ps2:
                    for b2 in range(B):
                        for sbl in range(S // 512):
                            sb = b2 * (S // 512) + sbl
                            xbh = xin.tile([128, EO, 512], bf16, tag="xbh")
                            nc.sync.dma_start(xbh, xTh[:, :, sb * 512:(sb + 1) * 512])
                            xbl = xin.tile([128, EO, 512], bf16, tag="xbl")
                            nc.sync.dma_start(xbl, xTl[:, :, sb * 512:(sb + 1) * 512])
                            for m in range(3):
                                ms = slice(m * 128, (m + 1) * 128)
                                ps = ps1.tile([128, 512], fp32)
                                if m == 2 and cfg_v1:
                                    for eo in range(EO):
                                        nc.tensor.matmul(
                                            ps, lhsT=w_ah_t[:, eo, ms],
                                            rhs=xbh[:, eo, :],
                                            start=(eo == 0), stop=(eo == EO - 1),
                                        )
                                else:
                                    for eo in range(EO):
                                        first, last = eo == 0, eo == EO - 1
                                        nc.tensor.matmul(
                                            ps, lhsT=w_ah_t[:, eo, ms], rhs=xbh[:, eo, :],
                                            start=first, stop=False,
                                        )
                                        nc.tensor.matmul(
                                            ps, lhsT=w_al_t[:, eo, ms], rhs=xbh[:, eo, :],
                                            start=False, stop=False,
                                        )
                                        nc.tensor.matmul(
                                            ps, lhsT=w_ah_t[:, eo, ms], rhs=xbl[:, eo, :],
                                            start=False, stop=last,
                                        )
                                dst = (q8[b2], k8[b2], v32T[b2])[m]
                                nc.scalar.activation(
                                    dst[:, sbl * 512:(sbl + 1) * 512], ps,
                                    AF.Identity, bias=b_a_t[:, m:m + 1],
                                )
                            # interleave E_mT load with qkv compute
                            nc.sync.dma_start(EmT[:, 2 * sb:2 * sb + 2, :],
                                              emT[:, 2 * sb:2 * sb + 2, :])
                            # P2: transpose this x-block's v (f32 -> fp8, 1 round)
                            for sc in (4 * sbl, 4 * sbl + 1, 4 * sbl + 2, 4 * sbl + 3):
                                ps = ps2.tile([128, 128], fp32)
                                nc.tensor.transpose(
                                    ps, v32T[b2][:, sc * 128: sc * 128 + 128],
                                    ident32,
                                )
                                nc.vector.tensor_copy(v8[b2][:, sc, :], ps)

            # ---- P4: attention, q-slabs of 1024, double-buffered ----
            QS = 1024
            with (
                tc.tile_pool(name="e_slab", bufs=2) as e_pool,
                tc.tile_pool(name="work", bufs=cfg_work) as work,
                tc.tile_pool(name="rwork", bufs=2) as rwork,
                tc.tile_pool(name="rw32", bufs=1) as rw32,
                tc.tile_pool(name="pout", bufs=3) as pout,
                tc.tile_pool(name="ps_s", bufs=cfg_ps_s, space="PSUM") as ps_s,
                tc.tile_pool(name="ps_z", bufs=1, space="PSUM") as ps_z,
                tc.tile_pool(name="ps_o", bufs=cfg_ps_o, space="PSUM") as ps_o,
            ):
                evac_state = [0]
                mk_i = [0]

                def scores_phase(b2, h, qh, e_sl):
                    hsl = h * D
                    q0 = qh * QS
                    # z accumulates alongside the scores stream: the ones-
                    # matmul for a chunk pair runs as soon as its gpsimd
                    # pre-sum lands.
                    psz = ps_z.tile([128, 2, 512], fp32, tag="zps")
                    for kp in range(KO // 2):
                        w8 = work.tile([128, 2, QS], fp8, tag="w8")
                        for kl in range(2):
                            kc = 2 * kp + kl
                            lhsT = k8[b2][hsl:hsl + D, kc * 128: kc * 128 + 128]
                            for qb in range(QS // 512):
                                ps = ps_s.tile([128, 512], fp32)
                                nc.tensor.matmul(
                                    ps, lhsT=lhsT,
                                    rhs=q8[b2][hsl:hsl + D,
                                               q0 + qb * 512: q0 + qb * 512 + 512],
                                    start=True, stop=True,
                                    perf_mode=(mybir.MatmulPerfMode.DoublePixel
                                               if cfg_dp else None),
                                )
                                if evac_state[0] % cfg_evac_d < cfg_evac_n:
                                    nc.scalar.copy(
                                        w8[:, kl, qb * 512:(qb + 1) * 512], ps)
                                else:
                                    nc.vector.tensor_copy(
                                        w8[:, kl, qb * 512:(qb + 1) * 512], ps)
                                evac_state[0] += 1
                        e_s = work.tile([128, 2, QS], fp16, tag="es")
                        nc.scalar.activation(e_s, w8, AF.Exp, scale=0.125)
                        if mk_i[0] % 8 < cfg_mk_gps:
                            nc.gpsimd.tensor_mul(
                                e_sl[:, 2 * kp:2 * kp + 2, :], e_s,
                                EmT[:, 2 * kp:2 * kp + 2, q0:q0 + QS],
                            )
                        else:
                            nc.vector.tensor_mul(
                                e_sl[:, 2 * kp:2 * kp + 2, :], e_s,
                                EmT[:, 2 * kp:2 * kp + 2, q0:q0 + QS],
                            )
                        mk_i[0] += 1
                        for kl in range(2):
                            kc = 2 * kp + kl
                            for qb in range(QS // 512):
                                nc.tensor.matmul(
                                    psz[:, qb, :], lhsT=ones128,
                                    rhs=e_sl[:, kc, qb * 512:(qb + 1) * 512],
                                    start=(kc == 0), stop=(kc == KO - 1),
                                )
                    rb32 = rw32.tile([128, 2, 512], fp32, tag="rb32")
                    nc.vector.reciprocal_approx_fast(rb32, psz)
                    rbf = rb32.rearrange("p a b -> p (a b)")
                    rbb = rwork.tile([128, 2, QS], fp16, tag="rb2")
                    nc.scalar.copy(rbb[:, 0, :], rbf)
                    nc.scalar.copy(rbb[:, 1, :], rbf)
                    return rbb

                def softmax_phase(b2, h, qh, e_sl, rbb):
                    hsl = h * D
                    q0g = b2 * S + qh * QS
                    pso = [ps_o.tile([D, 512], fp32, name=f"pso{i}", tag="pso")
                           for i in range(QS // 512)]
                    for j in range(KO // 2):
                        a8 = work.tile([128, 2, QS], fp8, tag="a8")
                        if cfg_gps_n and j % cfg_gps_d < cfg_gps_n:
                            nc.gpsimd.tensor_mul(a8, e_sl[:, 2 * j:2 * j + 2, :], rbb)
                        else:
                            nc.vector.tensor_mul(a8, e_sl[:, 2 * j:2 * j + 2, :], rbb)
                        for qb in range(QS // 512):
                            nc.tensor.matmul(
                                pso[qb],
                                lhsT=v8[b2][:, 2 * j:2 * j + 2, hsl:hsl + D],
                                rhs=a8[:, :, qb * 512:(qb + 1) * 512],
                                start=(j == 0), stop=(j == KO // 2 - 1),
                                perf_mode=mybir.MatmulPerfMode.DoubleRow,
                            )
                    for qb in range(QS // 512):
                        nc.vector.tensor_copy(
                            outT[hsl:hsl + D,
                                 q0g + qb * 512: q0g + (qb + 1) * 512],
                            pso[qb],
                        )

                def proj_slab(b2, qh):
                    # project the completed outT columns for this q-slab
                    s0 = b2 * S + qh * QS
                    for mt in range(s0 // 128, (s0 + QS) // 128):
                        for nb in range(E // 512):
                            ps = ps_s.tile([128, 512], fp32)
                            nc.tensor.matmul(
                                ps,
                                lhsT=outT[:, mt * 128:(mt + 1) * 128],
                                rhs=w_pT_t[:, nb * 512:(nb + 1) * 512],
                                start=True, stop=True,
                            )
                            ot = pout.tile([128, 512], fp16)
                            if (mt + nb) % 2 == 0:
                                nc.scalar.copy(ot, ps)
                            else:
                                nc.vector.tensor_copy(ot, ps)
                            nc.sync.dma_start(
                                out_d[mt * 128:(mt + 1) * 128,
                                      nb * 512:(nb + 1) * 512],
                                ot,
                            )

                units = [(b2, qh, h)
                         for b2 in range(B) for qh in range(S // QS)
                         for h in range(HPC)]
                prev = None
                for (b2, qh, h) in units:
                    e_sl = e_pool.tile([128, KO, QS], fp16, tag="e_sl")
                    rbb = scores_phase(b2, h, qh, e_sl)
                    if prev is not None:
                        softmax_phase(prev[0], prev[2], prev[1], prev[3], prev[4])
                        if prev[2] == HPC - 1:
                            proj_slab(prev[0], prev[1])
                    prev = (b2, qh, h, e_sl, rbb)
                softmax_phase(prev[0], prev[2], prev[1], prev[3], prev[4])
                proj_slab(prev[0], prev[1])

    nc.compile()
    return nc


_nc_cache = None


def host_prep(x, attn_mask, w_qkv, b_qkv, w_proj, b_proj):
    x = np.ascontiguousarray(np.asarray(x, dtype=np.float32))
    attn_mask = np.asarray(attn_mask, dtype=np.float32)
    w_qkv = np.asarray(w_qkv, dtype=np.float32)
    b_qkv = np.asarray(b_qkv, dtype=np.float32)
    w_proj = np.asarray(w_proj, dtype=np.float32)

    # host-side layout prep (sharding)
    xT = np.ascontiguousarray(x.reshape(BS, E).T)            # [E, BS]
    xT_dev = np.ascontiguousarray(xT.reshape(EO, 128, BS).transpose(1, 0, 2))
    xTh_dev = xT_dev.astype(ml_dtypes.bfloat16)
    xTl_dev = (xT_dev - xTh_dev.astype(np.float32)).astype(ml_dtypes.bfloat16)
    maskT = np.ascontiguousarray(attn_mask[0, 0].T)          # [S(k), S(q)] -> [k, q]
    m8T = maskT.astype(ml_dtypes.float8_e4m3fn).astype(np.float32)
    emT = np.exp(m8T).astype(np.float16)
    emT_dev = np.ascontiguousarray(emT.reshape(KO, 128, S).transpose(1, 0, 2))

    in_maps = []
    for c in range(NCORES):
        h0, h1 = HPC * c, HPC * c + 1
        rows = []
        for off in (0, E, 2 * E):   # q, k, v blocks of w_qkv
            rows += list(range(off + h0 * D, off + h0 * D + D))
            rows += list(range(off + h1 * D, off + h1 * D + D))
        rows = np.array(rows)
        w_a = w_qkv[rows]                                     # [384, 1024]
        w_aT = np.ascontiguousarray(w_a.T)                    # [1024, 384]
        w_a_dev = np.ascontiguousarray(w_aT.reshape(EO, 128, 384).transpose(1, 0, 2))
        w_ah_dev = w_a_dev.astype(ml_dtypes.bfloat16)
        w_al_dev = (w_a_dev - w_ah_dev.astype(np.float32)).astype(ml_dtypes.bfloat16)
        b_a_dev = np.ascontiguousarray(b_qkv[rows].reshape(3, 128).T)
        cols = np.concatenate([np.arange(h0 * D, h0 * D + D),
                               np.arange(h1 * D, h1 * D + D)])
        w_pT_dev = np.ascontiguousarray(w_proj[:, cols].T).astype(np.float16)
        in_maps.append({
            "xTh": xTh_dev, "xTl": xTl_dev, "emT": emT_dev,
            "w_ah": w_ah_dev, "w_al": w_al_dev,
            "b_a": b_a_dev, "w_pT": w_pT_dev,
        })

    return in_maps


def kernel(x, attn_mask, w_qkv, b_qkv, w_proj, b_proj):
    global _last_results, _nc_cache
    from concourse.bass_utils import run_bass_kernel_spmd

    in_maps = host_prep(x, attn_mask, w_qkv, b_qkv, w_proj, b_proj)
    b_proj = np.asarray(b_proj, dtype=np.float32)

    if _nc_cache is None:
        _nc_cache = _build_bass()
    nc = _nc_cache

    res = run_bass_kernel_spmd(
        nc, in_maps, core_ids=list(range(NCORES)),
        trace=bool(int(os.environ.get("KERNEL_TRACE", "0"))),
    )
    _last_results = res

    total = res.results[0]["out_p"].astype(np.float64)
    for c in range(1, NCORES):
        total = total + res.results[c]["out_p"].astype(np.float64)
    out = (total.astype(np.float32) + b_proj).reshape(B, S, E)
    return out


# revision 53
# speedup vs baseline: 1.2417x; 1.2417x over previous
"""Trainium2 Bass kernel for DecomposedQkvAttention (fp8-qdq attention).

Sharding: 16 heads / 8 cores = 2 heads per core (both batches on every core).
Each core computes qkv projection for its head slice, fp8-qdq attention in a
"transposed" layout (scores as [k_part, q_free]), and its partial contribution
to the output projection; the host sums the 8 partials and adds b_proj.

v4 (on the v0 schedule skeleton):
  - qkv: q,k in 3-pass bf16 hi/lo (fp8-flip sensitivity needs ~fp32 there),
    v in 1-pass bf16 (its flip noise averages out through the attention sum)
  - softmax z via all-ones [128,128] matmul (broadcasts z to every psum
    partition); 1/z via reciprocal_approx_fast (1 DVE op, ~18 bits)
  - attn@V in fp8 DoubleRow perf mode
  - outT / w_proj / output partials in fp16

Self-contained: hardcodes B=2, S=2048, E=1024, H=16, D=64.
"""
import os
import numpy as np
import ml_dtypes

B, S, E, H = 2, 2048, 1024, 16
D = E // H            # 64
NCORES = 8
HPC = H // NCORES     # heads per core = 2
BS = B * S            # 4096
KO = S // 128         # 16 k-chunks
QB = S // 512         # 4 q-blocks of 512
EO = E // 128         # 8 contraction chunks for qkv proj

_last_results = None  # stash for test harness (exec_time_ns etc.)


def _build_bass():
    import concourse.bass as bass
    import concourse.mybir as mybir
    import concourse.tile as tile
    from concourse import bacc
    from concourse.masks import make_identity

    fp32 = mybir.dt.float32
    bf16 = mybir.dt.bfloat16
    fp16 = mybir.dt.float16
    fp8 = mybir.dt.float8e4
    AF = mybir.ActivationFunctionType

    cfg_evac_n = int(os.environ.get("K_EVAC_N", "6"))    # ACT share numerator
    cfg_evac_d = int(os.environ.get("K_EVAC_D", "8"))    # denominator
    cfg_ps_s = int(os.environ.get("K_PS_S", "3"))
    cfg_ps_o = int(os.environ.get("K_PS_O", "2"))
    cfg_work = int(os.environ.get("K_WORK", "5"))
    cfg_gps_n = int(os.environ.get("K_GPS_N", "0"))      # a8 gps share n/d
    cfg_gps_d = int(os.environ.get("K_GPS_D", "3"))
    cfg_mk_gps = int(os.environ.get("K_MK_GPS", "0"))    # mask-mul gps of 8
    cfg_v1 = bool(int(os.environ.get("K_V1", "1")))      # v in 1-pass bf16
    cfg_dp = bool(int(os.environ.get("K_DP", "0")))      # DoublePixel scores

    nc = bacc.Bacc(
        "TRN2", target_bir_lowering=False, debug=False, num_devices=NCORES,
    )

    xTh = nc.dram_tensor("xTh", [128, EO, BS], bf16, kind="ExternalInput")
    xTl = nc.dram_tensor("xTl", [128, EO, BS], bf16, kind="ExternalInput")
    emT = nc.dram_tensor("emT", [128, KO, S], fp16, kind="ExternalInput")
    w_ah = nc.dram_tensor("w_ah", [128, EO, 3 * 128], bf16, kind="ExternalInput")
    w_al = nc.dram_tensor("w_al", [128, EO, 3 * 128], bf16, kind="ExternalInput")
    b_a = nc.dram_tensor("b_a", [128, 3], fp32, kind="ExternalInput")
    w_pT = nc.dram_tensor("w_pT", [128, E], fp16, kind="ExternalInput")
    out_d = nc.dram_tensor("out_p", [BS, E], fp16, kind="ExternalOutput")

    with tile.TileContext(nc) as tc:
        with tc.tile_pool(name="static", bufs=1) as static:
            w_pT_t = static.tile([128, E], fp16)
            nc.sync.dma_start(w_pT_t, w_pT[:])
            ones128 = static.tile([128, 128], fp16)
            nc.vector.memset(ones128, 1.0)

            # persistent activations (split per batch to unlock P1/P4 overlap)
            q8 = [static.tile([128, S], fp8, name=f"q8_{b}") for b in range(B)]
            k8 = [static.tile([128, S], fp8, name=f"k8_{b}") for b in range(B)]
            v8 = [static.tile([128, KO, 128], fp8, name=f"v8_{b}") for b in range(B)]
            EmT = static.tile([128, KO, S], fp16)     # exp(fp8(mask^T))
            outT = static.tile([128, BS], fp16)       # [(h,d), b*s]

            # ---- P1: qkv projection ----
            with (
                tc.tile_pool(name="xin", bufs=3) as xin,
                tc.tile_pool(name="v32", bufs=1) as v32p,
                tc.tile_pool(name="ps1", bufs=2, space="PSUM") as ps1,
            ):
                w_ah_t = v32p.tile([128, EO, 3 * 128], bf16)
                nc.sync.dma_start(w_ah_t, w_ah[:])
                w_al_t = v32p.tile([128, EO, 3 * 128], bf16)
                nc.sync.dma_start(w_al_t, w_al[:])
                b_a_t = v32p.tile([128, 3], fp32)
                nc.sync.dma_start(b_a_t, b_a[:])
                ident32 = v32p.tile([128, 128], fp32)
                make_identity(nc, ident32)
                v32T = [v32p.tile([128, S], fp32, name=f"v32T_{b}") for b in range(B)]
                with tc.tile_pool(name="ps2", bufs=2, space="PSUM") as ps2:
                    for b2 in range(B):
                        for sbl in range(S // 512):
                            sb = b2 * (S // 512) + sbl
                            xbh = xin.tile([128, EO, 512], bf16, tag="xbh")
                            nc.sync.dma_start(xbh, xTh[:, :, sb * 512:(sb + 1) * 512])
                            xbl = xin.tile([128, EO, 512], bf16, tag="xbl")
                            nc.sync.dma_start(xbl, xTl[:, :, sb * 512:(sb + 1) * 512])
                            for m in range(3):
                                ms = slice(m * 128, (m + 1) * 128)
                                ps = ps1.tile([128, 512], fp32)
                                if m == 2 and cfg_v1:
                                    for eo in range(EO):
                                        nc.tensor.matmul(
                                            ps, lhsT=w_ah_t[:, eo, ms],
                                            rhs=xbh[:, eo, :],
                                            start=(eo == 0), stop=(eo == EO - 1),
                                        )
                                else:
                                    for eo in range(EO):
                                        first, last = eo == 0, eo == EO - 1
                                        nc.tensor.matmul(
                                            ps, lhsT=w_ah_t[:, eo, ms], rhs=xbh[:, eo, :],
                                            start=first, stop=False,
                                        )
                                        nc.tensor.matmul(
                                            ps, lhsT=w_al_t[:, eo, ms], rhs=xbh[:, eo, :],
                                            start=False, stop=False,
                                        )
                                        nc.tensor.matmul(
                                            ps, lhsT=w_ah_t[:, eo, ms], rhs=xbl[:, eo, :],
                                            start=False, stop=last,
                                        )
                                dst = (q8[b2], k8[b2], v32T[b2])[m]
                                nc.scalar.activation(
                                    dst[:, sbl * 512:(sbl + 1) * 512], ps,
                                    AF.Identity, bias=b_a_t[:, m:m + 1],
                                )
                            # interleave E_mT load with qkv compute
                            nc.sync.dma_start(EmT[:, 2 * sb:2 * sb + 2, :],
                                              emT[:, 2 * sb:2 * sb + 2, :])
                            # P2: transpose this x-block's v (f32 -> fp8, 1 round)
                            for sc in (4 * sbl, 4 * sbl + 1, 4 * sbl + 2, 4 * sbl + 3):
                                ps = ps2.tile([128, 128], fp32)
                                nc.tensor.transpose(
                                    ps, v32T[b2][:, sc * 128: sc * 128 + 128],
                                    ident32,
                                )
                                nc.vector.tensor_copy(v8[b2][:, sc, :], ps)

            # ---- P4: attention, q-slabs of 1024, double-buffered ----
            QS = 1024
            with (
                tc.tile_pool(name="e_slab", bufs=2) as e_pool,
                tc.tile_pool(name="work", bufs=cfg_work) as work,
                tc.tile_pool(name="rwork", bufs=2) as rwork,
                tc.tile_pool(name="rw32", bufs=1) as rw32,
                tc.tile_pool(name="pout", bufs=3) as pout,
                tc.tile_pool(name="ps_s", bufs=cfg_ps_s, space="PSUM") as ps_s,
                tc.tile_pool(name="ps_z", bufs=1, space="PSUM") as ps_z,
                tc.tile_pool(name="ps_o", bufs=cfg_ps_o, space="PSUM") as ps_o,
                tc.tile_pool(name="ps_p", bufs=1, space="PSUM") as ps_p,
            ):
                evac_state = [0]
                mk_i = [0]

                def scores_phase(b2, h, qh, e_sl):
                    hsl = h * D
                    q0 = qh * QS
                    # z accumulates alongside the scores stream: the ones-
                    # matmul for a chunk pair runs as soon as its gpsimd
                    # pre-sum lands.
                    psz = ps_z.tile([128, 2, 512], fp32, tag="zps")
                    for kp in range(KO // 2):
                        w8 = work.tile([128, 2, QS], fp8, tag="w8")
                        for kl in range(2):
                            kc = 2 * kp + kl
                            lhsT = k8[b2][hsl:hsl + D, kc * 128: kc * 128 + 128]
                            for qb in range(QS // 512):
                                ps = ps_s.tile([128, 512], fp32)
                                nc.tensor.matmul(
                                    ps, lhsT=lhsT,
                                    rhs=q8[b2][hsl:hsl + D,
                                               q0 + qb * 512: q0 + qb * 512 + 512],
                                    start=True, stop=True,
                                    perf_mode=(mybir.MatmulPerfMode.DoublePixel
                                               if cfg_dp else None),
                                )
                                if evac_state[0] % cfg_evac_d < cfg_evac_n:
                                    nc.scalar.copy(
                                        w8[:, kl, qb * 512:(qb + 1) * 512], ps)
                                else:
                                    nc.vector.tensor_copy(
                                        w8[:, kl, qb * 512:(qb + 1) * 512], ps)
                                evac_state[0] += 1
                        e_s = work.tile([128, 2, QS], fp16, tag="es")
                        nc.scalar.activation(e_s, w8, AF.Exp, scale=0.125)
                        if mk_i[0] % 8 < cfg_mk_gps:
                            nc.gpsimd.tensor_mul(
                                e_sl[:, 2 * kp:2 * kp + 2, :], e_s,
                                EmT[:, 2 * kp:2 * kp + 2, q0:q0 + QS],
                            )
                        else:
                            nc.vector.tensor_mul(
                                e_sl[:, 2 * kp:2 * kp + 2, :], e_s,
                                EmT[:, 2 * kp:2 * kp + 2, q0:q0 + QS],
                            )
                        mk_i[0] += 1
                        # z for pair kp-1: one kp slot behind the mask-mul so
                        # the tensor queue never waits on the DVE here
                        if kp > 0:
                            for kl in range(2):
                                kc = 2 * (kp - 1) + kl
                                for qb in range(QS // 512):
                                    nc.tensor.matmul(
                                        psz[:, qb, :], lhsT=ones128,
                                        rhs=e_sl[:, kc, qb * 512:(qb + 1) * 512],
                                        start=(kc == 0), stop=False,
                                    )
                    for kl in range(2):
                        kc = 2 * (KO // 2 - 1) + kl
                        for qb in range(QS // 512):
                            nc.tensor.matmul(
                                psz[:, qb, :], lhsT=ones128,
                                rhs=e_sl[:, kc, qb * 512:(qb + 1) * 512],
                                start=False, stop=(kc == KO - 1),
                            )
                    rb32 = rw32.tile([128, 2, 512], fp32, tag="rb32")
                    nc.vector.reciprocal_approx_fast(rb32, psz)
                    rbf = rb32.rearrange("p a b -> p (a b)")
                    rbb = rwork.tile([128, 2, QS], fp16, tag="rb2")
                    nc.scalar.copy(rbb[:, 0, :], rbf)
                    nc.scalar.copy(rbb[:, 1, :], rbf)
                    return rbb

                def softmax_phase(b2, h, qh, e_sl, rbb):
                    hsl = h * D
                    q0g = b2 * S + qh * QS
                    pso = [ps_o.tile([D, 512], fp32, name=f"pso{i}", tag="pso")
                           for i in range(QS // 512)]
                    for j in range(KO // 2):
                        a8 = work.tile([128, 2, QS], fp8, tag="a8")
                        if cfg_gps_n and j % cfg_gps_d < cfg_gps_n:
                            nc.gpsimd.tensor_mul(a8, e_sl[:, 2 * j:2 * j + 2, :], rbb)
                        else:
                            nc.vector.tensor_mul(a8, e_sl[:, 2 * j:2 * j + 2, :], rbb)
                        for qb in range(QS // 512):
                            nc.tensor.matmul(
                                pso[qb],
                                lhsT=v8[b2][:, 2 * j:2 * j + 2, hsl:hsl + D],
                                rhs=a8[:, :, qb * 512:(qb + 1) * 512],
                                start=(j == 0), stop=(j == KO // 2 - 1),
                                perf_mode=mybir.MatmulPerfMode.DoubleRow,
                            )
                    for qb in range(QS // 512):
                        nc.vector.tensor_copy(
                            outT[hsl:hsl + D,
                                 q0g + qb * 512: q0g + (qb + 1) * 512],
                            pso[qb],
                        )

                def proj_slab(b2, qh):
                    # project the completed outT columns for this q-slab
                    s0 = b2 * S + qh * QS
                    for mt in range(s0 // 128, (s0 + QS) // 128):
                        for nb in range(E // 512):
                            ps = ps_p.tile([128, 512], fp32)
                            nc.tensor.matmul(
                                ps,
                                lhsT=outT[:, mt * 128:(mt + 1) * 128],
                                rhs=w_pT_t[:, nb * 512:(nb + 1) * 512],
                                start=True, stop=True,
                            )
                            ot = pout.tile([128, 512], fp16)
                            if (mt + nb) % 2 == 0:
                                nc.scalar.copy(ot, ps)
                            else:
                                nc.vector.tensor_copy(ot, ps)
                            nc.sync.dma_start(
                                out_d[mt * 128:(mt + 1) * 128,
                                      nb * 512:(nb + 1) * 512],
                                ot,
                            )

                units = [(b2, qh, h)
                         for b2 in range(B) for qh in range(S // QS)
                         for h in range(HPC)]
                prev = None
                for (b2, qh, h) in units:
                    e_sl = e_pool.tile([128, KO, QS], fp16, tag="e_sl")
                    rbb = scores_phase(b2, h, qh, e_sl)
                    if prev is not None:
                        softmax_phase(prev[0], prev[2], prev[1], prev[3], prev[4])
                        if prev[2] == HPC - 1:
                            proj_slab(prev[0], prev[1])
                    prev = (b2, qh, h, e_sl, rbb)
                softmax_phase(prev[0], prev[2], prev[1], prev[3], prev[4])
                proj_slab(prev[0], prev[1])

    nc.compile()
    return nc


_nc_cache = None


def host_prep(x, attn_mask, w_qkv, b_qkv, w_proj, b_proj):
    x = np.ascontiguousarray(np.asarray(x, dtype=np.float32))
    attn_mask = np.asarray(attn_mask, dtype=np.float32)
    w_qkv = np.asarray(w_qkv, dtype=np.float32)
    b_qkv = np.asarray(b_qkv, dtype=np.float32)
    w_proj = np.asarray(w_proj, dtype=np.float32)

    # host-side layout prep (sharding)
    xT = np.ascontiguousarray(x.reshape(BS, E).T)            # [E, BS]
    xT_dev = np.ascontiguousarray(xT.reshape(EO, 128, BS).transpose(1, 0, 2))
    xTh_dev = xT_dev.astype(ml_dtypes.bfloat16)
    xTl_dev = (xT_dev - xTh_dev.astype(np.float32)).astype(ml_dtypes.bfloat16)
    maskT = np.ascontiguousarray(attn_mask[0, 0].T)          # [S(k), S(q)] -> [k, q]
    m8T = maskT.astype(ml_dtypes.float8_e4m3fn).astype(np.float32)
    emT = np.exp(m8T).astype(np.float16)
    emT_dev = np.ascontiguousarray(emT.reshape(KO, 128, S).transpose(1, 0, 2))

    in_maps = []
    for c in range(NCORES):
        h0, h1 = HPC * c, HPC * c + 1
        rows = []
        for off in (0, E, 2 * E):   # q, k, v blocks of w_qkv
            rows += list(range(off + h0 * D, off + h0 * D + D))
            rows += list(range(off + h1 * D, off + h1 * D + D))
        rows = np.array(rows)
        w_a = w_qkv[rows]                                     # [384, 1024]
        w_aT = np.ascontiguousarray(w_a.T)                    # [1024, 384]
        w_a_dev = np.ascontiguousarray(w_aT.reshape(EO, 128, 384).transpose(1, 0, 2))
        w_ah_dev = w_a_dev.astype(ml_dtypes.bfloat16)
        w_al_dev = (w_a_dev - w_ah_dev.astype(np.float32)).astype(ml_dtypes.bfloat16)
        b_a_dev = np.ascontiguousarray(b_qkv[rows].reshape(3, 128).T)
        cols = np.concatenate([np.arange(h0 * D, h0 * D + D),
                               np.arange(h1 * D, h1 * D + D)])
        w_pT_dev = np.ascontiguousarray(w_proj[:, cols].T).astype(np.float16)
        in_maps.append({
            "xTh": xTh_dev, "xTl": xTl_dev, "emT": emT_dev,
            "w_ah": w_ah_dev, "w_al": w_al_dev,
            "b_a": b_a_dev, "w_pT": w_pT_dev,
        })

    return in_maps


def kernel(x, attn_mask, w_qkv, b_qkv, w_proj, b_proj):
    global _last_results, _nc_cache
    from concourse.bass_utils import run_bass_kernel_spmd

    in_maps = host_prep(x, attn_mask, w_qkv, b_qkv, w_proj, b_proj)
    b_proj = np.asarray(b_proj, dtype=np.float32)

    if _nc_cache is None:
        _nc_cache = _build_bass()
    nc = _nc_cache

    res = run_bass_kernel_spmd(
        nc, in_maps, core_ids=list(range(NCORES)),
        trace=bool(int(os.environ.get("KERNEL_TRACE", "0"))),
    )
    _last_results = res

    total = res.results[0]["out_p"].astype(np.float64)
    for c in range(1, NCORES):
        total = total + res.results[c]["out_p"].astype(np.float64)
    out = (total.astype(np.float32) + b_proj).reshape(B, S, E)
    return out
